# revision 81
# baseline (speedup 1.0000x reference)
"""Trainium2 Bass kernel for nn_Attention_31215822307478.

EfficientViT-style attention block:
  qkv 1x1 conv + BN -> split q,k,v -> depthwise 3x3 + BN on q ->
  8-head attention with positional bias over N=784 tokens ->
  ReLU -> 1x1 proj + BN.

Strategy (per core, data-parallel over batch, 4 images/core):
  - BN folded into conv weights/biases on host.
  - k produced directly in head-padded layout (each 16-ch head at a
    32-aligned partition offset) by ordering the conv's stationary columns;
    q produced unpadded so the depthwise conv runs once over 128 channels,
    then scattered into the padded QK layout by 8 tiny SBUF->SBUF DMAs on
    the otherwise idle sync queue.
  - Depthwise 3x3 via 9 accumulated diagonal matmuls over a zero-padded
    [128, 30x30] buffer (single unpadded group: half the padded cost).
  - V produced pre-transposed ([spatial, ch]) straight from the 1x1 conv by
    swapping matmul operand roles; the per-head softmax-denominator ones
    column is written by the evacuation op (the conv computes data only).
  - Softmax is max-free (logits provably small); positional bias applied as
    a host-precomputed exp(bias) table multiply, split DVE (kt 0-3) /
    GpSimd (kt 4-6) to balance engine load.
  - AV matmuls are software-pipelined one key-tile behind QK/exp so the PE
    stream never stalls on the A operand: sub-microsecond PE idle keeps the
    HAM clock-gate from dropping the PE to half rate mid-pair.
  - Image b+1's x DMA issues at the top of image b (latency hidden); its
    conv/depthwise matmul burst issues after image b's pairs, exactly where
    the normalize/proj dependency hole would otherwise idle the PE.
  - Z rows gathered at 32-strided partitions of two tiles, one
    reciprocal_approx_fast each (~5x faster than reciprocal), rounded to
    f32r, then per-pair f32r ones-matmuls (1 cycle/row vs 4 for fp32)
    broadcast 1/Z across partitions in PSUM for the divide.
  - rt tiles pair-packed [128, N] so the proj matmul contracts 128 rows.
  - Deep E-tile ring (14) so exp never waits on the slower GpSimd bias
    multiplies through buffer-reuse hazards.
"""

import os
import sys

import numpy as np

for _p in ("/opt/trn_rl_repo", "/root/.axon_site/_ro/trn_rl_repo"):
    if os.path.isdir(_p) and _p not in sys.path:
        sys.path.insert(0, _p)

import ml_dtypes  # noqa: E402
from contextlib import ExitStack  # noqa: E402

import concourse.bass as bass  # noqa: E402
import concourse.mybir as mybir  # noqa: E402
import concourse.tile as tile  # noqa: E402
from concourse import bacc  # noqa: E402
from concourse.alu_op_type import AluOpType  # noqa: E402
from concourse.bass_utils import run_bass_kernel_spmd  # noqa: E402

EPS = 1e-5
DIM, KEY_DIM, HEADS = 256, 16, 8
NH_KD, D, DH = 128, 64, 512
B, H, W = 32, 28, 28
N = H * W  # 784
NCORES = 8
BC = B // NCORES  # 4 images per core
SCALE = KEY_DIM ** -0.5

F32 = mybir.dt.float32
F32R = mybir.dt.float32r
BF16 = mybir.dt.bfloat16
FP8 = mybir.dt.float8e4
DR = mybir.MatmulPerfMode.DoubleRow
AF = mybir.ActivationFunctionType

KT_SIZES = [128] * 6 + [16]  # 784 = 6*128 + 16 key tiles
CH = [(0, 512), (512, 272)]  # PSUM-bank-aligned free chunks of 784


def bias_engine(h, kt):
    """Which engine applies the positional bias for (head, key-tile)."""
    if kt in (0, 1, 2, 3):
        return "dve"
    return "pool"  # kt in (4, 5, 6)


_PROGRAM_CACHE = {}


def _build_program():
    nc = bacc.Bacc("TRN2", target_bir_lowering=False, debug=False)

    x_d = nc.dram_tensor("x", [BC, 256, N], BF16, kind="ExternalInput").ap()
    wqkT_d = nc.dram_tensor("wqkT", [2, 128, 384], BF16, kind="ExternalInput").ap()
    wvT_d = nc.dram_tensor("wvT", [2, 128, 512], BF16, kind="ExternalInput").ap()
    wpT_d = nc.dram_tensor("wpT", [4, 128, 256], BF16, kind="ExternalInput").ap()
    dtap_d = nc.dram_tensor("dtaps", [128, 9 * 128], BF16, kind="ExternalInput").ap()
    bias_d = nc.dram_tensor("biases", [128, 8], F32, kind="ExternalInput").ap()
    bvb_d = nc.dram_tensor("bv_bcast", [128, 520], BF16, kind="ExternalInput").ap()
    eb_d = nc.dram_tensor("eb", [128, HEADS * 7 * N], BF16, kind="ExternalInput").ap()
    zsel_d = nc.dram_tensor("zsel", [128, 128], F32, kind="ExternalInput").ap()
    ones1_d = nc.dram_tensor("ones1", [1, N], F32, kind="ExternalInput").ap()
    out_d = nc.dram_tensor("out", [BC, 256, N], F32, kind="ExternalOutput").ap()

    with tile.TileContext(nc) as tc, ExitStack() as ctx:
        const = ctx.enter_context(tc.tile_pool(name="const", bufs=1))
        pspool = ctx.enter_context(tc.tile_pool(name="ps", bufs=2, space="PSUM"))
        upool = ctx.enter_context(tc.tile_pool(name="ups", bufs=2, space="PSUM"))
        xpool = ctx.enter_context(tc.tile_pool(name="xp", bufs=3))
        qpadp = ctx.enter_context(tc.tile_pool(name="qpadp", bufs=2))
        kpool = ctx.enter_context(tc.tile_pool(name="kp", bufs=4))
        qdpool = ctx.enter_context(tc.tile_pool(name="qdp", bufs=2))
        qspool = ctx.enter_context(tc.tile_pool(name="qsp", bufs=4))
        vpool = ctx.enter_context(tc.tile_pool(name="vp", bufs=14))
        epool = ctx.enter_context(tc.tile_pool(name="ep", bufs=14))
        apool = ctx.enter_context(tc.tile_pool(name="ap", bufs=5))
        rpool = ctx.enter_context(tc.tile_pool(name="rp", bufs=2))
        zspool = ctx.enter_context(tc.tile_pool(name="zsp", bufs=2))
        rtpool = ctx.enter_context(tc.tile_pool(name="rtp", bufs=8))
        opool = ctx.enter_context(tc.tile_pool(name="op", bufs=2))

        # ---- constants (ordered so image-0 critical path loads first) ----
        biases = const.tile([128, 8], F32, tag="biases", name="biases")
        nc.sync.dma_start(biases[:], bias_d[:])
        wqkT = []
        for ki in range(2):
            t = const.tile([128, 384], BF16, tag=f"wqkT{ki}", name=f"wqkT{ki}")
            nc.sync.dma_start(t[:], wqkT_d[ki])
            wqkT.append(t)
        # declared now, loaded in stages below
        wvT = []
        for ki in range(2):
            wvT.append(const.tile([128, 512], BF16, tag=f"wvT{ki}", name=f"wvT{ki}"))
        bvb = const.tile([128, 520], BF16, tag="bvb", name="bvb")
        dtap = const.tile([128, 9 * 128], BF16, tag="dtap", name="dtap")
        zsel = const.tile([128, 128], F32, tag="zsel", name="zsel")
        zselr = const.tile([128, 128], F32R, tag="zselr", name="zselr")
        ones1 = const.tile([1, N], F32, tag="ones1", name="ones1")
        wpT = [
            const.tile([128, 256], BF16, tag=f"wpT{ki}", name=f"wpT{ki}")
            for ki in range(4)
        ]
        eb = const.tile([128, HEADS * 7 * N], BF16, tag="eb", name="eb")

        st = [dict() for _ in range(BC)]  # per-image tile state

        def phase_a_dma(b):
            s = st[b]
            s["xb"] = []
            for ki in range(2):
                t = xpool.tile([128, N], BF16, tag="xb", name="xb")
                nc.sync.dma_start(t[:], x_d[b, 128 * ki : 128 * (ki + 1), :])
                s["xb"].append(t)
            qp = qpadp.tile([128, 900], BF16, tag="qpad", name="qpad")
            nc.gpsimd.memset(qp[:], 0.0)
            s["q_pad"] = qp

        def phase_a_chunk(b, c, sps=range(7)):
            s = st[b]
            if c == 0:
                # qk 1x1 conv (mt0 = q unpadded, mt1/2 = k padded)
                qp = s["q_pad"]
                s["kds"] = []
                for mt in range(3):
                    ps = pspool.tile([128, N], F32, tag="ps", name="ps")
                    for (o, szc) in CH:
                        for ki in range(2):
                            nc.tensor.matmul(
                                ps[:, o : o + szc],
                                wqkT[ki][:, mt * 128 : (mt + 1) * 128],
                                s["xb"][ki][:, o : o + szc],
                                start=(ki == 0),
                                stop=(ki == 1),
                            )
                    if mt == 0:
                        qp3 = qp[:].rearrange("p (y x) -> p y x", y=30)
                        nc.vector.tensor_scalar_add(
                            qp3[:, 1:29, 1:29],
                            ps[:].rearrange("p (y x) -> p y x", y=28),
                            biases[:, 0:1],
                        )
                    else:
                        kt_ = kpool.tile([128, N], BF16, tag="ksb", name="ksb")
                        nc.vector.tensor_scalar_add(
                            kt_[:], ps[:], biases[:, mt : mt + 1]
                        )
                        s["kds"].append(kt_)
            elif c == 1:
                # v 1x1 conv, transposed output [spatial, 8*(64+1)]
                s.setdefault("vt", [])
                for sp in sps:
                    ssz = KT_SIZES[sp]
                    psv = pspool.tile([128, 512], F32, tag="ps", name="ps")
                    for ki in range(2):
                        nc.tensor.matmul(
                            psv[:ssz, :],
                            s["xb"][ki][:, sp * 128 : sp * 128 + ssz],
                            wvT[ki][:, :],
                            start=(ki == 0),
                            stop=(ki == 1),
                        )
                    vtt = vpool.tile([128, 520], BF16, tag="vt", name="vt")
                    vt3 = vtt[:].rearrange("p (h c) -> p h c", h=8)
                    bv3 = bvb[:].rearrange("p (h c) -> p h c", h=8)
                    nc.vector.tensor_tensor(
                        vt3[:ssz, :, 0:64],
                        psv[:ssz, :].rearrange("p (h c) -> p h c", h=8),
                        bv3[:ssz, :, 0:64],
                        op=AluOpType.add,
                    )
                    nc.vector.tensor_scalar_add(
                        vt3[:ssz, :, 64:65], bv3[:ssz, :, 64:65], 0.0
                    )
                    s["vt"].append(vtt)
            else:
                # depthwise 3x3 via 9 diagonal matmuls (single unpadded
                # group), then scatter the result into the padded QK layout
                # via 8 tiny SBUF->SBUF DMAs.
                qp3 = s["q_pad"][:].rearrange("p (y x) -> p y x", y=30)
                psd = pspool.tile([128, 1024], F32, tag="ps", name="ps")
                for (y0, ny, po) in [(0, 14, 0), (14, 14, 512)]:
                    for t9 in range(9):
                        ty, tx = divmod(t9, 3)
                        nc.tensor.matmul(
                            psd[:, po : po + ny * 28],
                            dtap[:, t9 * 128 : (t9 + 1) * 128],
                            qp3[:, y0 + ty : y0 + ty + ny, tx : tx + 28],
                            start=(t9 == 0),
                            stop=(t9 == 8),
                        )
                qd = qdpool.tile([128, N], BF16, tag="qdw", name="qdw")
                nc.vector.tensor_scalar_add(
                    qd[:, 0:392], psd[:, 0:392], biases[:, 5:6]
                )
                nc.vector.tensor_scalar_add(
                    qd[:, 392:784], psd[:, 512:904], biases[:, 5:6]
                )
                s["qds"] = [
                    qspool.tile([128, N], BF16, tag="qds", name="qds")
                    for _ in range(2)
                ]
                for h in range(HEADS):
                    g, j = divmod(h, 4)
                    nc.sync.dma_start(
                        s["qds"][g][32 * j : 32 * j + 16, :],
                        qd[16 * h : 16 * h + 16, :],
                    )

        def _issue_av(s, hs, Us, kt, As):
            ksz = KT_SIZES[kt]
            for hi, h in enumerate(hs):
                for (o, szc) in CH:
                    nc.tensor.matmul(
                        Us[hi][:, o : o + szc],
                        s["vt"][kt][:ksz, 65 * h : 65 * h + 65],
                        As[hi][:ksz, o : o + szc],
                        start=(kt == 0),
                        stop=(kt == 6),
                    )

        def phase_b_flush(b, relus=True):
            # The final AV (kt6) of a pair plus the Z/relu evacuation is
            # deferred into the next pair (emitted after its first QK batch)
            # so the ACT engine gets a head start on the new pair's exps and
            # the PE never stalls >1us at the seam (which would drop the HAM
            # clock to half rate).
            s = st[b]
            if "pend" not in s:
                return
            hs, Us, As, rt = s.pop("pend")
            _issue_av(s, hs, Us, 6, As)
            for hi, h in enumerate(hs):
                g2, j2 = divmod(h, 4)
                nc.vector.scalar_tensor_tensor(
                    s["zs"][g2][32 * j2 : 32 * j2 + 1, :],
                    Us[hi][64:65, :],
                    0.0,
                    ones1[:],
                    op0=AluOpType.add,
                    op1=AluOpType.mult,
                )
            if relus:
                for hi in range(2):
                    nc.vector.tensor_scalar_max(
                        rt[64 * hi : 64 * hi + 64, :], Us[hi][0:64, :], 0.0
                    )
            else:
                s["defer"] = (rt, Us)

        def phase_b_pair(b, hp):
            s = st[b]
            if hp == 0:
                s["zs"] = []
                for g2 in range(2):
                    zt = zspool.tile([128, N], F32, tag="zs", name="zs")
                    nc.gpsimd.memset(zt[:], 1.0)
                    s["zs"].append(zt)
                s["rt"] = []
            rt = rtpool.tile([128, N], BF16, tag="rt", name="rt")
            s["rt"].append(rt)
            hs = (2 * hp, 2 * hp + 1)
            Us = [upool.tile([65, N], F32, tag="U", name="U") for _ in hs]
            prevA = None
            for kt in range(7):
                ksz = KT_SIZES[kt]
                Ss = []
                for hi, h in enumerate(hs):
                    g, j = divmod(h, 4)
                    base = 32 * j
                    S = pspool.tile([128, N], F32, tag="ps", name="ps")
                    for (o, szc) in CH:
                        nc.tensor.matmul(
                            S[:ksz, o : o + szc],
                            s["kds"][g][base : base + 16, kt * 128 : kt * 128 + ksz],
                            s["qds"][g][base : base + 16, o : o + szc],
                            start=True,
                            stop=True,
                            tile_position=(base, 0),
                        )
                    Ss.append(S)
                if prevA is not None:
                    _issue_av(s, hs, Us, kt - 1, prevA)
                As = []
                for hi, h in enumerate(hs):
                    E = epool.tile([128, N], BF16, tag="E", name="E")
                    nc.scalar.activation(
                        E[:ksz, :], Ss[hi][:ksz, :], AF.Exp, scale=SCALE
                    )
                    A = apool.tile([128, N], BF16, tag="A", name="A")
                    tt_eng = nc.gpsimd if bias_engine(h, kt) == "pool" else nc.vector
                    tt_eng.tensor_tensor(
                        A[:ksz, :],
                        E[:ksz, :],
                        eb[:ksz, (h * 7 + kt) * N : (h * 7 + kt + 1) * N],
                        op=AluOpType.mult,
                    )
                    As.append(A)
                prevA = As
            s["pend"] = (hs, Us, prevA, rt)

        def phase_b_relu_deferred(b):
            rt, Us = st[b].pop("defer")
            for hi in range(2):
                nc.vector.tensor_scalar_max(
                    rt[64 * hi : 64 * hi + 64, :], Us[hi][0:64, :], 0.0
                )

        def phase_b_div_recip(b):
            s = st[b]
            s["rz"] = []
            for g2 in range(2):
                rzt = rpool.tile([128, N], F32, tag="rz", name="rz")
                nc.vector.reciprocal_approx_fast(rzt[:], s["zs"][g2][:])
                rzr = rpool.tile([128, N], F32R, tag="rzr", name="rzr")
                nc.vector.tensor_scalar_add(rzr[:], rzt[:], 0.0)
                s["rz"].append(rzr)

        def phase_b_div_pair(b, hp):
            s = st[b]
            g2, e = divmod(hp, 2)
            Rb = pspool.tile([128, N], F32, tag="ps", name="ps")
            for (o, szc) in CH:
                nc.tensor.matmul(
                    Rb[:, o : o + szc],
                    zselr[64 * e : 64 * e + 33, :],
                    s["rz"][g2][64 * e : 64 * e + 33, o : o + szc],
                    start=True,
                    stop=True,
                    tile_position=(64 * e, 0),
                )
            nc.vector.tensor_tensor(
                s["rt"][hp][:], s["rt"][hp][:], Rb[:], op=AluOpType.mult
            )

        def phase_c(b):
            s = st[b]
            for mt in range(2):
                po_ = pspool.tile([128, N], F32, tag="ps", name="ps")
                for (o, szc) in CH:
                    for ki in range(4):
                        nc.tensor.matmul(
                            po_[:, o : o + szc],
                            wpT[ki][:, mt * 128 : (mt + 1) * 128],
                            s["rt"][ki][:, o : o + szc],
                            start=(ki == 0),
                            stop=(ki == 3),
                        )
                ob = opool.tile([128, N], F32, tag="ob", name="ob")
                nc.vector.tensor_scalar_add(ob[:], po_[:], biases[:, 3 + mt : 4 + mt])
                nc.sync.dma_start(out_d[b, mt * 128 : (mt + 1) * 128, :], ob[:])

        # image 0's convs interleave with the remaining const loads so the
        # first QK conv only waits on biases/wqkT/x; the 11MB bias tables ride
        # the gpsimd DMA queue afterwards.
        phase_a_dma(0)
        phase_a_chunk(0, 0)
        for ki in range(2):
            nc.sync.dma_start(wvT[ki][:], wvT_d[ki])
        nc.sync.dma_start(bvb[:], bvb_d[:])
        phase_a_chunk(0, 1)
        nc.sync.dma_start(dtap[:], dtap_d[:])
        phase_a_chunk(0, 2)
        nc.sync.dma_start(zsel[:], zsel_d[:])
        nc.vector.tensor_scalar_add(zselr[:], zsel[:], 0.0)
        nc.sync.dma_start(ones1[:], ones1_d[:])
        for ki in range(4):
            nc.sync.dma_start(wpT[ki][:], wpT_d[ki])
        for h in range(HEADS):
            nc.gpsimd.dma_start(
                eb[:, h * 7 * N : (h + 1) * 7 * N],
                eb_d[:, h * 7 * N : (h + 1) * 7 * N],
            )
        for b in range(BC):
            if b + 1 < BC:
                phase_a_dma(b + 1)
            for hp in range(4):
                phase_b_pair(b, hp)
                if hp < 3:
                    phase_b_flush(b)
            phase_b_flush(b, relus=False)
            if b + 1 < BC:
                for c in range(3):
                    phase_a_chunk(b + 1, c)
            phase_b_div_recip(b)
            phase_b_relu_deferred(b)
            for hp in range(4):
                phase_b_div_pair(b, hp)
            phase_c(b)

    nc.compile()
    return nc


def get_program():
    if "nc" not in _PROGRAM_CACHE:
        _PROGRAM_CACHE["nc"] = _build_program()
    return _PROGRAM_CACHE["nc"]


def prep_host_inputs(inputs):
    """Fold BN, reorder weights, build bias tables. Returns dict of np arrays
    for the non-x DRAM tensors (shared across cores)."""
    f32 = np.float32
    bf = ml_dtypes.bfloat16
    qkv_w = np.asarray(inputs["qkv_w"], f32)[:, :, 0, 0]  # [768, 256]
    s = np.asarray(inputs["qkv_g"], f32) / np.sqrt(np.asarray(inputs["qkv_v"], f32) + EPS)
    Wall = qkv_w * s[:, None]
    ball = np.asarray(inputs["qkv_b"], f32) - np.asarray(inputs["qkv_m"], f32) * s
    Wq, Wk, Wv = Wall[:128], Wall[128:256], Wall[256:]
    bq, bk, bv = ball[:128], ball[128:256], ball[256:]

    # mt0 = q natural; mt1/mt2 = k padded (head h at 32*(h%4) of group h//4)
    Wk_pad = np.zeros((256, 256), f32)
    bk_pad = np.zeros(256, f32)
    for h in range(HEADS):
        g, j = divmod(h, 4)
        r = 128 * g + 32 * j
        Wk_pad[r : r + 16] = Wk[16 * h : 16 * h + 16]
        bk_pad[r : r + 16] = bk[16 * h : 16 * h + 16]
    W3 = np.concatenate([Wq, Wk_pad], axis=0)  # [384 out, 256 in]
    wqkT = np.ascontiguousarray(W3.T).reshape(2, 128, 384)

    # wvT: data-only (8 heads x 64 ch contiguous); the interleaved ones
    # column of the vt layout is written by the evacuation op instead.
    wvT = np.ascontiguousarray(Wv.T).reshape(2, 128, 512)
    bv_aug = np.zeros(520, f32)
    for h in range(HEADS):
        bv_aug[65 * h : 65 * h + 64] = bv[64 * h : 64 * h + 64]
        bv_aug[65 * h + 64] = 1.0
    bv_bcast = np.ascontiguousarray(np.broadcast_to(bv_aug, (128, 520)))

    s2 = np.asarray(inputs["dw_g"], f32) / np.sqrt(np.asarray(inputs["dw_v"], f32) + EPS)
    dww = np.asarray(inputs["dw_w"], f32)[:, 0] * s2[:, None, None]  # [128,3,3]
    bdw = np.asarray(inputs["dw_b"], f32) - np.asarray(inputs["dw_m"], f32) * s2
    dtaps = np.zeros((128, 9 * 128), f32)
    for c in range(128):
        for t9 in range(9):
            dtaps[c, t9 * 128 + c] = dww[c].reshape(9)[t9]

    sp = np.asarray(inputs["proj_g"], f32) / np.sqrt(
        np.asarray(inputs["proj_v"], f32) + EPS
    )
    Wp = np.asarray(inputs["proj_w"], f32)[:, :, 0, 0] * sp[:, None]  # [256, 512]
    bp = np.asarray(inputs["proj_b"], f32) - np.asarray(inputs["proj_m"], f32) * sp
    wpT = np.ascontiguousarray(Wp.T).reshape(4, 128, 256)

    biases = np.zeros((128, 8), f32)
    biases[:, 0] = bq
    biases[:, 1] = bk_pad[:128]
    biases[:, 2] = bk_pad[128:]
    biases[:, 3] = bp[:128]
    biases[:, 4] = bp[128:]
    biases[:, 5] = bdw

    ab = np.asarray(inputs["ab"], f32)  # [8, 784]
    idx = np.asarray(inputs["bias_idxs"])  # [784, 784] int32
    Bm = ab[:, idx]  # [8, key, query] (bias is symmetric)
    eb = np.zeros((128, HEADS * 7 * N), f32)
    for h in range(HEADS):
        for kt in range(7):
            ksz = KT_SIZES[kt]
            blk = np.exp(Bm[h, kt * 128 : kt * 128 + ksz, :])
            eb[:ksz, (h * 7 + kt) * N : (h * 7 + kt + 1) * N] = blk

    zsel = np.zeros((128, 128), f32)
    for r in (0, 64):
        zsel[r, 0:64] = 1.0
    for r in (32, 96):
        zsel[r, 64:128] = 1.0

    return {
        "zsel": zsel,
        "ones1": np.ones((1, N), f32),
        "wqkT": wqkT.astype(bf),
        "wvT": wvT.astype(bf),
        "wpT": wpT.astype(bf),
        "dtaps": dtaps.astype(bf),
        "biases": biases,
        "bv_bcast": bv_bcast.astype(bf),
        "eb": eb.astype(bf),
    }


def kernel(**inputs):
    nc = get_program()
    shared = prep_host_inputs(inputs)
    x = np.asarray(inputs["x"], np.float32).reshape(B, 256, N).astype(ml_dtypes.bfloat16)
    in_maps = []
    for c in range(NCORES):
        m = dict(shared)
        m["x"] = np.ascontiguousarray(x[BC * c : BC * (c + 1)])
        in_maps.append(m)
    res = run_bass_kernel_spmd(nc, in_maps, core_ids=list(range(NCORES)))
    out = np.concatenate([r["out"] for r in res.results], axis=0)
    return out.reshape(B, 256, H, W)


# revision 82
# speedup vs baseline: 1.0378x; 1.0378x over previous
"""Trainium2 Bass kernel for nn_Attention_31215822307478.

EfficientViT-style attention block:
  qkv 1x1 conv + BN -> split q,k,v -> depthwise 3x3 + BN on q ->
  8-head attention with positional bias over N=784 tokens ->
  ReLU -> 1x1 proj + BN.

Strategy (per core, data-parallel over batch, 4 images/core):
  - BN folded into conv weights/biases on host.
  - k produced directly in head-padded layout (each 16-ch head at a
    32-aligned partition offset) by ordering the conv's stationary columns;
    q produced unpadded so the depthwise conv runs once over 128 channels,
    then scattered into the padded QK layout by 8 tiny SBUF->SBUF DMAs on
    the otherwise idle sync queue.
  - Depthwise 3x3 via 9 accumulated diagonal matmuls over a zero-padded
    [128, 30x30] buffer (single unpadded group: half the padded cost).
  - V produced pre-transposed ([spatial, ch]) straight from the 1x1 conv by
    swapping matmul operand roles; the per-head softmax-denominator ones
    column is written by the evacuation op (the conv computes data only).
  - Softmax is max-free (logits provably small); positional bias applied as
    a host-precomputed exp(bias) table multiply, split DVE (kt 0-3) /
    GpSimd (kt 4-6) to balance engine load.
  - AV matmuls are software-pipelined one key-tile behind QK/exp so the PE
    stream never stalls on the A operand: sub-microsecond PE idle keeps the
    HAM clock-gate from dropping the PE to half rate mid-pair.
  - Image b+1's x DMA issues at the top of image b (latency hidden); its
    conv/depthwise matmul burst issues after image b's pairs, exactly where
    the normalize/proj dependency hole would otherwise idle the PE.
  - Z rows gathered at 32-strided partitions of two tiles, one
    reciprocal_approx_fast each (~5x faster than reciprocal), rounded to
    f32r, then per-pair f32r ones-matmuls (1 cycle/row vs 4 for fp32)
    broadcast 1/Z across partitions in PSUM for the divide.
  - rt tiles pair-packed [128, N] so the proj matmul contracts 128 rows.
  - Deep E-tile ring (14) so exp never waits on the slower GpSimd bias
    multiplies through buffer-reuse hazards.
"""

import os
import sys

import numpy as np

for _p in ("/opt/trn_rl_repo", "/root/.axon_site/_ro/trn_rl_repo"):
    if os.path.isdir(_p) and _p not in sys.path:
        sys.path.insert(0, _p)

import ml_dtypes  # noqa: E402
from contextlib import ExitStack  # noqa: E402

import concourse.bass as bass  # noqa: E402
import concourse.mybir as mybir  # noqa: E402
import concourse.tile as tile  # noqa: E402
from concourse import bacc  # noqa: E402
from concourse.alu_op_type import AluOpType  # noqa: E402
from concourse.bass_utils import run_bass_kernel_spmd  # noqa: E402

EPS = 1e-5
DIM, KEY_DIM, HEADS = 256, 16, 8
NH_KD, D, DH = 128, 64, 512
B, H, W = 32, 28, 28
N = H * W  # 784
NCORES = 8
BC = B // NCORES  # 4 images per core
SCALE = KEY_DIM ** -0.5

F32 = mybir.dt.float32
F32R = mybir.dt.float32r
BF16 = mybir.dt.bfloat16
FP8 = mybir.dt.float8e4
DR = mybir.MatmulPerfMode.DoubleRow
AF = mybir.ActivationFunctionType

KT_SIZES = [128] * 6 + [16]  # 784 = 6*128 + 16 key tiles
CH = [(0, 512), (512, 272)]  # PSUM-bank-aligned free chunks of 784


def bias_engine(h, kt):
    """Which engine applies the positional bias for (head, key-tile)."""
    if kt in (0, 1, 2, 3):
        return "dve"
    return "pool"  # kt in (4, 5, 6)


_PROGRAM_CACHE = {}


def _build_program():
    nc = bacc.Bacc("TRN2", target_bir_lowering=False, debug=False)

    x_d = nc.dram_tensor("x", [BC, 256, N], BF16, kind="ExternalInput").ap()
    wqkT_d = nc.dram_tensor("wqkT", [2, 128, 384], BF16, kind="ExternalInput").ap()
    wvT_d = nc.dram_tensor("wvT", [2, 128, 512], BF16, kind="ExternalInput").ap()
    wpT_d = nc.dram_tensor("wpT", [4, 128, 256], BF16, kind="ExternalInput").ap()
    dtap_d = nc.dram_tensor("dtaps", [128, 9 * 128], BF16, kind="ExternalInput").ap()
    bias_d = nc.dram_tensor("biases", [128, 8], F32, kind="ExternalInput").ap()
    bvb_d = nc.dram_tensor("bv_bcast", [128, 520], BF16, kind="ExternalInput").ap()
    eb_d = nc.dram_tensor("eb", [128, HEADS * 7 * N], BF16, kind="ExternalInput").ap()
    zsel_d = nc.dram_tensor("zsel", [128, 128], F32, kind="ExternalInput").ap()
    ones1_d = nc.dram_tensor("ones1", [1, N], F32, kind="ExternalInput").ap()
    out_d = nc.dram_tensor("out", [BC, 256, N], F32, kind="ExternalOutput").ap()

    with tile.TileContext(nc) as tc, ExitStack() as ctx:
        const = ctx.enter_context(tc.tile_pool(name="const", bufs=1))
        pspool = ctx.enter_context(tc.tile_pool(name="ps", bufs=2, space="PSUM"))
        upool = ctx.enter_context(tc.tile_pool(name="ups", bufs=2, space="PSUM"))
        xpool = ctx.enter_context(tc.tile_pool(name="xp", bufs=3))
        qpadp = ctx.enter_context(tc.tile_pool(name="qpadp", bufs=2))
        kpool = ctx.enter_context(tc.tile_pool(name="kp", bufs=4))
        qdpool = ctx.enter_context(tc.tile_pool(name="qdp", bufs=2))
        qspool = ctx.enter_context(tc.tile_pool(name="qsp", bufs=4))
        vpool = ctx.enter_context(tc.tile_pool(name="vp", bufs=14))
        epool = ctx.enter_context(tc.tile_pool(name="ep", bufs=14))
        apool = ctx.enter_context(tc.tile_pool(name="ap", bufs=5))
        rpool = ctx.enter_context(tc.tile_pool(name="rp", bufs=2))
        zspool = ctx.enter_context(tc.tile_pool(name="zsp", bufs=2))
        rtpool = ctx.enter_context(tc.tile_pool(name="rtp", bufs=8))
        opool = ctx.enter_context(tc.tile_pool(name="op", bufs=2))

        # ---- constants (ordered so image-0 critical path loads first) ----
        biases = const.tile([128, 8], F32, tag="biases", name="biases")
        nc.sync.dma_start(biases[:], bias_d[:])
        wqkT = []
        for ki in range(2):
            t = const.tile([128, 384], BF16, tag=f"wqkT{ki}", name=f"wqkT{ki}")
            nc.sync.dma_start(t[:], wqkT_d[ki])
            wqkT.append(t)
        # declared now, loaded in stages below
        wvT = []
        for ki in range(2):
            wvT.append(const.tile([128, 512], BF16, tag=f"wvT{ki}", name=f"wvT{ki}"))
        bvb = const.tile([128, 520], BF16, tag="bvb", name="bvb")
        dtap = const.tile([128, 9 * 128], BF16, tag="dtap", name="dtap")
        zsel = const.tile([128, 128], F32, tag="zsel", name="zsel")
        zselr = const.tile([128, 128], F32R, tag="zselr", name="zselr")
        ones1 = const.tile([1, N], F32, tag="ones1", name="ones1")
        wpT = [
            const.tile([128, 256], BF16, tag=f"wpT{ki}", name=f"wpT{ki}")
            for ki in range(4)
        ]
        eb = const.tile([128, HEADS * 7 * N], BF16, tag="eb", name="eb")

        st = [dict() for _ in range(BC)]  # per-image tile state

        def phase_a_dma(b):
            s = st[b]
            s["xb"] = []
            for ki in range(2):
                t = xpool.tile([128, N], BF16, tag="xb", name="xb")
                nc.sync.dma_start(t[:], x_d[b, 128 * ki : 128 * (ki + 1), :])
                s["xb"].append(t)

        def phase_a_chunk(b, c, sps=range(7)):
            s = st[b]
            if c == 0:
                # qk 1x1 conv (mt0 = q unpadded, mt1/2 = k padded)
                qp = qpadp.tile([128, 900], BF16, tag="qpad", name="qpad")
                nc.gpsimd.memset(qp[:], 0.0)
                s["q_pad"] = qp
                s["kds"] = []
                for mt in range(3):
                    ps = pspool.tile([128, N], F32, tag="ps", name="ps")
                    for (o, szc) in CH:
                        for ki in range(2):
                            nc.tensor.matmul(
                                ps[:, o : o + szc],
                                wqkT[ki][:, mt * 128 : (mt + 1) * 128],
                                s["xb"][ki][:, o : o + szc],
                                start=(ki == 0),
                                stop=(ki == 1),
                            )
                    if mt == 0:
                        qp3 = qp[:].rearrange("p (y x) -> p y x", y=30)
                        nc.vector.tensor_scalar_add(
                            qp3[:, 1:29, 1:29],
                            ps[:].rearrange("p (y x) -> p y x", y=28),
                            biases[:, 0:1],
                        )
                    else:
                        kt_ = kpool.tile([128, N], BF16, tag="ksb", name="ksb")
                        nc.vector.tensor_scalar_add(
                            kt_[:], ps[:], biases[:, mt : mt + 1]
                        )
                        s["kds"].append(kt_)
            elif c == 1:
                # v 1x1 conv, transposed output [spatial, 8*(64+1)]
                s.setdefault("vt", [])
                for sp in sps:
                    ssz = KT_SIZES[sp]
                    psv = pspool.tile([128, 512], F32, tag="ps", name="ps")
                    for ki in range(2):
                        nc.tensor.matmul(
                            psv[:ssz, :],
                            s["xb"][ki][:, sp * 128 : sp * 128 + ssz],
                            wvT[ki][:, :],
                            start=(ki == 0),
                            stop=(ki == 1),
                        )
                    vtt = vpool.tile([128, 520], BF16, tag="vt", name="vt")
                    vt3 = vtt[:].rearrange("p (h c) -> p h c", h=8)
                    bv3 = bvb[:].rearrange("p (h c) -> p h c", h=8)
                    nc.vector.tensor_tensor(
                        vt3[:ssz, :, 0:64],
                        psv[:ssz, :].rearrange("p (h c) -> p h c", h=8),
                        bv3[:ssz, :, 0:64],
                        op=AluOpType.add,
                    )
                    nc.vector.tensor_scalar_add(
                        vt3[:ssz, :, 64:65], bv3[:ssz, :, 64:65], 0.0
                    )
                    s["vt"].append(vtt)
            else:
                # depthwise 3x3 via 9 diagonal matmuls (single unpadded
                # group), then scatter the result into the padded QK layout
                # via 8 tiny SBUF->SBUF DMAs.
                qp3 = s["q_pad"][:].rearrange("p (y x) -> p y x", y=30)
                psd = pspool.tile([128, 1024], F32, tag="ps", name="ps")
                for (y0, ny, po) in [(0, 14, 0), (14, 14, 512)]:
                    for t9 in range(9):
                        ty, tx = divmod(t9, 3)
                        nc.tensor.matmul(
                            psd[:, po : po + ny * 28],
                            dtap[:, t9 * 128 : (t9 + 1) * 128],
                            qp3[:, y0 + ty : y0 + ty + ny, tx : tx + 28],
                            start=(t9 == 0),
                            stop=(t9 == 8),
                        )
                qd = qdpool.tile([128, N], BF16, tag="qdw", name="qdw")
                nc.vector.tensor_scalar_add(
                    qd[:, 0:392], psd[:, 0:392], biases[:, 5:6]
                )
                nc.vector.tensor_scalar_add(
                    qd[:, 392:784], psd[:, 512:904], biases[:, 5:6]
                )
                s["qds"] = [
                    qspool.tile([128, N], BF16, tag="qds", name="qds")
                    for _ in range(2)
                ]
                for h in range(HEADS):
                    g, j = divmod(h, 4)
                    nc.sync.dma_start(
                        s["qds"][g][32 * j : 32 * j + 16, :],
                        qd[16 * h : 16 * h + 16, :],
                    )

        def _issue_av(s, hs, Us, kt, As):
            ksz = KT_SIZES[kt]
            for hi, h in enumerate(hs):
                for (o, szc) in CH:
                    nc.tensor.matmul(
                        Us[hi][:, o : o + szc],
                        s["vt"][kt][:ksz, 65 * h : 65 * h + 65],
                        As[hi][:ksz, o : o + szc],
                        start=(kt == 0),
                        stop=(kt == 6),
                    )

        def phase_b_flush(b, relus=True):
            # The final AV (kt6) of a pair plus the Z/relu evacuation is
            # deferred into the next pair (emitted after its first QK batch)
            # so the ACT engine gets a head start on the new pair's exps and
            # the PE never stalls >1us at the seam (which would drop the HAM
            # clock to half rate).
            s = st[b]
            if "pend" not in s:
                return
            hs, Us, As, rt = s.pop("pend")
            _issue_av(s, hs, Us, 6, As)
            for hi, h in enumerate(hs):
                g2, j2 = divmod(h, 4)
                nc.vector.scalar_tensor_tensor(
                    s["zs"][g2][32 * j2 : 32 * j2 + 1, :],
                    Us[hi][64:65, :],
                    0.0,
                    ones1[:],
                    op0=AluOpType.add,
                    op1=AluOpType.mult,
                )
            if relus:
                for hi in range(2):
                    nc.vector.tensor_scalar_max(
                        rt[64 * hi : 64 * hi + 64, :], Us[hi][0:64, :], 0.0
                    )
            else:
                s["defer"] = (rt, Us)

        def phase_b_pair(b, hp):
            s = st[b]
            if hp == 0:
                s["zs"] = []
                for g2 in range(2):
                    zt = zspool.tile([128, N], F32, tag="zs", name="zs")
                    nc.gpsimd.memset(zt[:], 1.0)
                    s["zs"].append(zt)
                s["rt"] = []
            rt = rtpool.tile([128, N], BF16, tag="rt", name="rt")
            s["rt"].append(rt)
            hs = (2 * hp, 2 * hp + 1)
            Us = [upool.tile([65, N], F32, tag="U", name="U") for _ in hs]
            prevA = None
            for kt in range(7):
                ksz = KT_SIZES[kt]
                Ss = []
                for hi, h in enumerate(hs):
                    g, j = divmod(h, 4)
                    base = 32 * j
                    S = pspool.tile([128, N], F32, tag="ps", name="ps")
                    for (o, szc) in CH:
                        nc.tensor.matmul(
                            S[:ksz, o : o + szc],
                            s["kds"][g][base : base + 16, kt * 128 : kt * 128 + ksz],
                            s["qds"][g][base : base + 16, o : o + szc],
                            start=True,
                            stop=True,
                            tile_position=(base, 0),
                        )
                    Ss.append(S)
                if prevA is not None:
                    _issue_av(s, hs, Us, kt - 1, prevA)
                As = []
                for hi, h in enumerate(hs):
                    E = epool.tile([128, N], BF16, tag="E", name="E")
                    nc.scalar.activation(
                        E[:ksz, :], Ss[hi][:ksz, :], AF.Exp, scale=SCALE
                    )
                    A = apool.tile([128, N], BF16, tag="A", name="A")
                    tt_eng = nc.gpsimd if bias_engine(h, kt) == "pool" else nc.vector
                    tt_eng.tensor_tensor(
                        A[:ksz, :],
                        E[:ksz, :],
                        eb[:ksz, (h * 7 + kt) * N : (h * 7 + kt + 1) * N],
                        op=AluOpType.mult,
                    )
                    As.append(A)
                prevA = As
            s["pend"] = (hs, Us, prevA, rt)

        def phase_b_relu_deferred(b):
            rt, Us = st[b].pop("defer")
            for hi in range(2):
                nc.vector.tensor_scalar_max(
                    rt[64 * hi : 64 * hi + 64, :], Us[hi][0:64, :], 0.0
                )

        def phase_b_div_recip(b):
            s = st[b]
            s["rz"] = []
            for g2 in range(2):
                rzt = rpool.tile([128, N], F32, tag="rz", name="rz")
                nc.vector.reciprocal_approx_fast(rzt[:], s["zs"][g2][:])
                rzr = rpool.tile([128, N], F32R, tag="rzr", name="rzr")
                nc.vector.tensor_scalar_add(rzr[:], rzt[:], 0.0)
                s["rz"].append(rzr)

        def phase_b_div_pair(b, hp):
            s = st[b]
            g2, e = divmod(hp, 2)
            Rb = pspool.tile([128, N], F32, tag="ps", name="ps")
            for (o, szc) in CH:
                nc.tensor.matmul(
                    Rb[:, o : o + szc],
                    zselr[64 * e : 64 * e + 33, :],
                    s["rz"][g2][64 * e : 64 * e + 33, o : o + szc],
                    start=True,
                    stop=True,
                    tile_position=(64 * e, 0),
                )
            nc.vector.tensor_tensor(
                s["rt"][hp][:], s["rt"][hp][:], Rb[:], op=AluOpType.mult
            )

        def phase_c(b):
            s = st[b]
            for mt in range(2):
                po_ = pspool.tile([128, N], F32, tag="ps", name="ps")
                for (o, szc) in CH:
                    for ki in range(4):
                        nc.tensor.matmul(
                            po_[:, o : o + szc],
                            wpT[ki][:, mt * 128 : (mt + 1) * 128],
                            s["rt"][ki][:, o : o + szc],
                            start=(ki == 0),
                            stop=(ki == 3),
                        )
                ob = opool.tile([128, N], F32, tag="ob", name="ob")
                nc.vector.tensor_scalar_add(ob[:], po_[:], biases[:, 3 + mt : 4 + mt])
                nc.sync.dma_start(out_d[b, mt * 128 : (mt + 1) * 128, :], ob[:])

        # image 0's convs interleave with the remaining const loads so the
        # first QK conv only waits on biases/wqkT/x; the 11MB bias tables ride
        # the gpsimd DMA queue afterwards.
        phase_a_dma(0)
        phase_a_chunk(0, 0)
        for ki in range(2):
            nc.sync.dma_start(wvT[ki][:], wvT_d[ki])
        nc.sync.dma_start(bvb[:], bvb_d[:])
        phase_a_chunk(0, 1)
        nc.sync.dma_start(dtap[:], dtap_d[:])
        phase_a_chunk(0, 2)
        nc.sync.dma_start(zsel[:], zsel_d[:])
        nc.vector.tensor_scalar_add(zselr[:], zsel[:], 0.0)
        nc.sync.dma_start(ones1[:], ones1_d[:])
        for ki in range(4):
            nc.sync.dma_start(wpT[ki][:], wpT_d[ki])
        for h in range(HEADS):
            nc.gpsimd.dma_start(
                eb[:, h * 7 * N : (h + 1) * 7 * N],
                eb_d[:, h * 7 * N : (h + 1) * 7 * N],
            )
        for b in range(BC):
            if b + 1 < BC:
                phase_a_dma(b + 1)
            for hp in range(4):
                phase_b_pair(b, hp)
                if hp < 3:
                    phase_b_flush(b)
            if b + 1 < BC:
                for c in range(3):
                    phase_a_chunk(b + 1, c)
            phase_b_flush(b, relus=False)
            phase_b_div_recip(b)
            phase_b_relu_deferred(b)
            for hp in range(4):
                phase_b_div_pair(b, hp)
            phase_c(b)

    nc.compile()
    return nc


def get_program():
    if "nc" not in _PROGRAM_CACHE:
        _PROGRAM_CACHE["nc"] = _build_program()
    return _PROGRAM_CACHE["nc"]


def prep_host_inputs(inputs):
    """Fold BN, reorder weights, build bias tables. Returns dict of np arrays
    for the non-x DRAM tensors (shared across cores)."""
    f32 = np.float32
    bf = ml_dtypes.bfloat16
    qkv_w = np.asarray(inputs["qkv_w"], f32)[:, :, 0, 0]  # [768, 256]
    s = np.asarray(inputs["qkv_g"], f32) / np.sqrt(np.asarray(inputs["qkv_v"], f32) + EPS)
    Wall = qkv_w * s[:, None]
    ball = np.asarray(inputs["qkv_b"], f32) - np.asarray(inputs["qkv_m"], f32) * s
    Wq, Wk, Wv = Wall[:128], Wall[128:256], Wall[256:]
    bq, bk, bv = ball[:128], ball[128:256], ball[256:]

    # mt0 = q natural; mt1/mt2 = k padded (head h at 32*(h%4) of group h//4)
    Wk_pad = np.zeros((256, 256), f32)
    bk_pad = np.zeros(256, f32)
    for h in range(HEADS):
        g, j = divmod(h, 4)
        r = 128 * g + 32 * j
        Wk_pad[r : r + 16] = Wk[16 * h : 16 * h + 16]
        bk_pad[r : r + 16] = bk[16 * h : 16 * h + 16]
    W3 = np.concatenate([Wq, Wk_pad], axis=0)  # [384 out, 256 in]
    wqkT = np.ascontiguousarray(W3.T).reshape(2, 128, 384)

    # wvT: data-only (8 heads x 64 ch contiguous); the interleaved ones
    # column of the vt layout is written by the evacuation op instead.
    wvT = np.ascontiguousarray(Wv.T).reshape(2, 128, 512)
    bv_aug = np.zeros(520, f32)
    for h in range(HEADS):
        bv_aug[65 * h : 65 * h + 64] = bv[64 * h : 64 * h + 64]
        bv_aug[65 * h + 64] = 1.0
    bv_bcast = np.ascontiguousarray(np.broadcast_to(bv_aug, (128, 520)))

    s2 = np.asarray(inputs["dw_g"], f32) / np.sqrt(np.asarray(inputs["dw_v"], f32) + EPS)
    dww = np.asarray(inputs["dw_w"], f32)[:, 0] * s2[:, None, None]  # [128,3,3]
    bdw = np.asarray(inputs["dw_b"], f32) - np.asarray(inputs["dw_m"], f32) * s2
    dtaps = np.zeros((128, 9 * 128), f32)
    for c in range(128):
        for t9 in range(9):
            dtaps[c, t9 * 128 + c] = dww[c].reshape(9)[t9]

    sp = np.asarray(inputs["proj_g"], f32) / np.sqrt(
        np.asarray(inputs["proj_v"], f32) + EPS
    )
    Wp = np.asarray(inputs["proj_w"], f32)[:, :, 0, 0] * sp[:, None]  # [256, 512]
    bp = np.asarray(inputs["proj_b"], f32) - np.asarray(inputs["proj_m"], f32) * sp
    wpT = np.ascontiguousarray(Wp.T).reshape(4, 128, 256)

    biases = np.zeros((128, 8), f32)
    biases[:, 0] = bq
    biases[:, 1] = bk_pad[:128]
    biases[:, 2] = bk_pad[128:]
    biases[:, 3] = bp[:128]
    biases[:, 4] = bp[128:]
    biases[:, 5] = bdw

    ab = np.asarray(inputs["ab"], f32)  # [8, 784]
    idx = np.asarray(inputs["bias_idxs"])  # [784, 784] int32
    Bm = ab[:, idx]  # [8, key, query] (bias is symmetric)
    eb = np.zeros((128, HEADS * 7 * N), f32)
    for h in range(HEADS):
        for kt in range(7):
            ksz = KT_SIZES[kt]
            blk = np.exp(Bm[h, kt * 128 : kt * 128 + ksz, :])
            eb[:ksz, (h * 7 + kt) * N : (h * 7 + kt + 1) * N] = blk

    zsel = np.zeros((128, 128), f32)
    for r in (0, 64):
        zsel[r, 0:64] = 1.0
    for r in (32, 96):
        zsel[r, 64:128] = 1.0

    return {
        "zsel": zsel,
        "ones1": np.ones((1, N), f32),
        "wqkT": wqkT.astype(bf),
        "wvT": wvT.astype(bf),
        "wpT": wpT.astype(bf),
        "dtaps": dtaps.astype(bf),
        "biases": biases,
        "bv_bcast": bv_bcast.astype(bf),
        "eb": eb.astype(bf),
    }


def kernel(**inputs):
    nc = get_program()
    shared = prep_host_inputs(inputs)
    x = np.asarray(inputs["x"], np.float32).reshape(B, 256, N).astype(ml_dtypes.bfloat16)
    in_maps = []
    for c in range(NCORES):
        m = dict(shared)
        m["x"] = np.ascontiguousarray(x[BC * c : BC * (c + 1)])
        in_maps.append(m)
    res = run_bass_kernel_spmd(nc, in_maps, core_ids=list(range(NCORES)))
    out = np.concatenate([r["out"] for r in res.results], axis=0)
    return out.reshape(B, 256, H, W)
